# revision 67
# baseline (speedup 1.0000x reference)
"""Trainium2 Bass kernel for causal multi-head attention block.

Reference computation (fp32):
    qkv = x @ w_qkv;  q,k,v = split(qkv)
    attn = softmax(causal_mask(q k^T / sqrt(64)))
    out  = (attn @ v reassembled) @ w_out

Sharding over 8 NeuronCores: core c handles batch b = c//4 and heads
4*(c%4) .. 4*(c%4)+3 (4 of 16 heads).  Each core computes the rank-256
partial product of the output projection restricted to its heads'
channels; the host sums the 4 partials per batch.

Projections (x @ w_qkv, out @ w_out) run as float32r (fp32 with 11-bit
mantissa, full-rate PE mode); the attention inner loops (q k^T, P v) run
in fp16, which enables fast weight load and full-rate K=64 row-group
packing.  Softmax skips the max-subtraction (logits are O(10), fp32 exp
is safe); denominators ride along as a fused 65th lhsT column, and the
reciprocal runs on a [128,8] reshape via a DRAM round trip with a
partition-broadcast DMA.  Measured ~195-205us per core on TRN2
(scale-relative max err ~3e-4 vs the fp32 reference).
"""

import sys

for _p in ("/opt/trn_rl_repo", "/root/.axon_site/_ro/trn_rl_repo"):
    if _p not in sys.path:
        sys.path.append(_p)

import numpy as np

import concourse.bass as bass
import concourse.mybir as mybir
import concourse.tile as tile
from concourse import bacc, bass_utils

P = 128
B, T, C = 2, 2048, 1024
HPC = 4            # heads per core
DH = 64            # head dim
KT = C // P        # 8 contraction tiles over d_model
NQB = T // 512     # 4 query blocks of 512
NKT = T // P       # 16 key tiles of 128
F32 = mybir.dt.float32
R32 = mybir.dt.float32r
F16 = mybir.dt.float16
EXP = mybir.ActivationFunctionType.Exp
SCALE = 1.0 / 8.0  # 1/sqrt(DH)


def _body(tc, nc, xT, wq, wk, wv, wo, tri, vones, out):
    with tc.tile_pool(name="const", bufs=1) as cpool:
        wq_sb = cpool.tile([P, KT, 2 * P], R32, name="wq_sb")
        wk_sb = cpool.tile([P, KT, 2 * P], R32, name="wk_sb")
        wv_sb = cpool.tile([P, KT, 2 * P], R32, name="wv_sb")
        wo_sb = cpool.tile([P, 2, C], R32, name="wo_sb")
        tri_sb = cpool.tile([P, P], F16, name="tri_sb")
        # halves, interleaved with the first x tile, so the first
        # accumulation chain starts early; bulky later-phase constants go
        # through the gpsimd (SWDGE) queue so they don't delay the critical
        # path.
        wqv = wq.rearrange("(kt p) n -> p kt n", p=P)
        wkv = wk.rearrange("(kt p) n -> p kt n", p=P)
        wvv = wv.rearrange("(kt p) n -> p kt n", p=P)
        nc.sync.dma_start(wq_sb[:, 0:1], wqv[:, 0:1])
        nc.sync.dma_start(wq_sb[:, 1:4], wqv[:, 1:4])
        nc.gpsimd.dma_start(wo_sb, wo.rearrange("(g p) n -> p g n", p=P))
        nc.gpsimd.dma_start(tri_sb, tri)

        # preload the exp ACT table set during the startup DMA window
        warm = cpool.tile([1, 2], F32, name="warm")
        nc.vector.memset(warm, 0.0)
        nc.scalar.activation(warm, warm, EXP, scale=1.0)

        # persistent stores
        qT = [cpool.tile([P, T], F16, name=f"qT{pr}") for pr in range(2)]
        kT = [cpool.tile([P, T], F16, name=f"kT{pr}") for pr in range(2)]
        # v with a fused ones column: [T-part, ktile, head, 65]
        vS = cpool.tile([P, NKT, HPC, DH + 1], F16, name="vS")
        nc.gpsimd.dma_start(vS[:, :, :, DH : DH + 1], vones)
        oT = [cpool.tile([P, T], R32, name=f"oT{pr}") for pr in range(2)]
        oTu = [cpool.tile([DH + 1, 2, T], F32, name=f"oTu{pr}") for pr in range(2)]

        # ---------- phase 1: q/k/v projections ----------
        xTv = xT.rearrange("(kt p) t -> p kt t", p=P)
        with (
            tc.tile_pool(name="xt", bufs=3) as xpool,
            tc.tile_pool(name="ps1", bufs=2, space="PSUM") as ps1,
            tc.tile_pool(name="vps", bufs=2, space="PSUM") as vps,
        ):
            for tb5 in range(NQB):
                xt = xpool.tile([P, KT, 512], R32, name="xt")
                if tb5 == 0:
                    # fine-grained first chunks: the q chains start after
                    # ~0.4MB of DMA; k/v weights stream in behind
                    nc.sync.dma_start(xt[:, 0:1, :], xTv[:, 0:1, 0:512])
                    nc.sync.dma_start(xt[:, 1:4, :], xTv[:, 1:4, 0:512])
                    nc.sync.dma_start(wq_sb[:, 4:8], wqv[:, 4:8])
                    nc.sync.dma_start(xt[:, 4:8, :], xTv[:, 4:8, 0:512])
                    nc.sync.dma_start(wk_sb[:, 0:4], wkv[:, 0:4])
                    nc.sync.dma_start(wk_sb[:, 4:8], wkv[:, 4:8])
                    nc.sync.dma_start(wv_sb, wvv)
                else:
                    nc.sync.dma_start(
                        xt, xTv[:, :, tb5 * 512 : (tb5 + 1) * 512]
                    )
                qps = [ps1.tile([P, 512], F32, name=f"qp{pr}", tag=f"qp{pr}", bufs=1) for pr in range(2)]
                kps = [ps1.tile([P, 512], F32, name=f"kp{pr}", tag=f"kp{pr}", bufs=1) for pr in range(2)]
                for kt in range(KT):
                    for pr in range(2):
                        nc.tensor.matmul(
                            qps[pr],
                            wq_sb[:, kt, pr * P : (pr + 1) * P],
                            xt[:, kt, :],
                            start=(kt == 0),
                            stop=(kt == KT - 1),
                        )
                for kt in range(KT):
                    for pr in range(2):
                        nc.tensor.matmul(
                            kps[pr],
                            wk_sb[:, kt, pr * P : (pr + 1) * P],
                            xt[:, kt, :],
                            start=(kt == 0),
                            stop=(kt == KT - 1),
                        )
                for pr in range(2):
                    nc.vector.tensor_copy(qT[pr][:, tb5 * 512 : (tb5 + 1) * 512], qps[pr])
                    nc.vector.tensor_copy(kT[pr][:, tb5 * 512 : (tb5 + 1) * 512], kps[pr])
                for sub in range(4):
                    tb1 = tb5 * 4 + sub
                    vp = vps.tile([P, 2 * P], F32, name="vp")
                    for kt in range(KT):
                        nc.tensor.matmul(
                            vp,
                            xt[:, kt, sub * P : (sub + 1) * P],
                            wv_sb[:, kt, :],
                            start=(kt == 0),
                            stop=(kt == KT - 1),
                        )
                    nc.vector.tensor_copy(
                        vS[:, tb1, :, 0:DH],
                        vp.rearrange("p (h d) -> p h d", d=DH),
                    )

        # ---------- phase 2: attention ----------
        # S^T blocks: [k-tile(128) x q-block(<=512)] per head; 2 heads packed
        # on PE row groups (contraction K=64 each).  exp via ScalarE.
        # O'^T = [v | 1]-style accumulation: O rows + denominator row in the
        # same PSUM bank, relying on per-element has_written semantics
        # (single start=True matmul per bank, everything else start=False).
        with (
            tc.tile_pool(name="sps", bufs=3, space="PSUM") as sps,
            tc.tile_pool(name="ops", bufs=1, space="PSUM") as ops,
            tc.tile_pool(name="ptp", bufs=3) as ptp,
            tc.tile_pool(name="nrm", bufs=2) as nrm,
            tc.tile_pool(name="dsc", bufs=2, space="DRAM") as dsc,
            tc.tile_pool(name="osb", bufs=3) as osb,
        ):
            def emit_proj_one(tb1, tag, use_act=False):
                # output projection for one T block (both pairs' oT final).
                pp = sps.tile([P, 1024], F32, name="pp", tag=tag)
                for cb in range(2):
                    for pr in range(2):
                        nc.tensor.matmul(
                            pp[:, cb * 512 : (cb + 1) * 512],
                            oT[pr][:, tb1 * P : (tb1 + 1) * P],
                            wo_sb[:, pr, cb * 512 : (cb + 1) * 512],
                            start=(pr == 0),
                            stop=(pr == 1),
                        )
                ot = osb.tile([P, 1024], F32, name="ot")
                if use_act:
                    nc.scalar.copy(ot[:, 0:512], pp[:, 0:512])
                else:
                    nc.vector.tensor_copy(ot[:, 0:512], pp[:, 0:512])
                nc.vector.tensor_copy(ot[:, 512:1024], pp[:, 512:1024])
                nc.sync.dma_start(out[tb1 * P : (tb1 + 1) * P, :], ot)

            qb_order = [1, 0, 2, 3]
            # spread proj(0) / proj(2) into the two long qb=3 chains (their
            # normalization chains drained at least one full group earlier)
            proj_sched = {}
            for i, tb1 in enumerate(range(0, 4)):
                proj_sched[(3, 0, 5 + 3 * i)] = tb1
            for i, tb1 in enumerate(range(8, 12)):
                proj_sched[(3, 1, 5 + 3 * i)] = tb1
            for qi, qb in enumerate(qb_order):
                for pr in range(2):
                    op = ops.tile([P, 1024], F32, name="op", tag="op")
                    nk = 4 * qb + 4

                    def geom(j):
                        r = j - 4 * qb
                        width = 512 - r * P if r >= 0 else 512
                        col0 = r * P if r >= 0 else 0
                        return r, width, col0

                    def emit_o(j, pts):
                        _, width, col0 = geom(j)
                        pt = pts.pop(j)
                        for h in range(2):
                            nc.tensor.matmul(
                                op[0 : DH + 1, h * 512 + col0 : (h + 1) * 512],
                                vS[:, j, pr * 2 + h, :],
                                pt[:, h * 512 : h * 512 + width],
                                start=(j == 0),
                                stop=(j == nk - 1),
                                skip_group_check=True,
                            )

                    pts = {}
                    for j in range(nk):
                        r, width, col0 = geom(j)
                        qoff = qb * 512 + col0
                        sp_ = sps.tile([P, 1024], F32, name="sp_", tag="sp")
                        for h in range(2):
                            nc.tensor.matmul(
                                sp_[:, h * 512 : h * 512 + width],
                                kT[pr][h * DH : (h + 1) * DH, j * P : (j + 1) * P],
                                qT[pr][h * DH : (h + 1) * DH, qoff : qoff + width],
                                start=True,
                                stop=True,
                            )
                        pt = ptp.tile([P, 1024], F16, name="pt")
                        s3 = sp_.rearrange("p (h w) -> p h w", h=2)[:, :, 0:width]
                        p3 = pt.rearrange("p (h w) -> p h w", h=2)[:, :, 0:width]
                        nc.scalar.activation(p3, s3, EXP, scale=SCALE)
                        if r >= 0:
                            # triangular mask on the first 128 valid columns
                            for h in range(2):
                                nc.vector.tensor_mul(
                                    pt[:, h * 512 : h * 512 + P],
                                    pt[:, h * 512 : h * 512 + P],
                                    tri_sb,
                                )
                        pts[j] = pt
                        # software pipeline: O' for step j-1 after S/exp of j
                        if j > 0:
                            emit_o(j - 1, pts)
                        if (qi, pr, j) in proj_sched:
                            emit_proj_one(proj_sched[(qi, pr, j)], "sp")
                    emit_o(nk - 1, pts)

                    # fast unnormalized evacuation: frees the op PSUM slot
                    # ~1.4us after the chain; the reciprocal/broadcast chain
                    # then runs against SBUF off the critical resource.
                    qs = slice(qb * 512, (qb + 1) * 512)
                    nc.vector.tensor_copy(
                        oTu[pr][:, :, qs],
                        op.rearrange("p (h w) -> p h w", h=2)[0 : DH + 1],
                    )
                    dd = dsc.tile([1024], F32, name="dd", tag="dd")
                    nc.sync.dma_start(
                        dd.rearrange("(h w) -> h w", h=2)[None],
                        oTu[pr][DH : DH + 1, :, qs],
                    )
                    rsh = nrm.tile([P, 8], F32, name="rsh", tag="rsh")
                    nc.sync.dma_start(rsh, dd.rearrange("(p c) -> p c", p=P))
                    rr = nrm.tile([P, 8], F32, name="rr", tag="rr")
                    nc.vector.reciprocal(rr, rsh)
                    dd2 = dsc.tile([1024], F32, name="dd2", tag="dd2")
                    nc.sync.dma_start(dd2.rearrange("(p c) -> p c", p=P), rr)
                    bc = nrm.tile([DH, 1024], F32, name="bc", tag="bc")
                    nc.sync.dma_start(
                        bc[:, 0:512], dd2[None, 0:512].to_broadcast([DH, 512])
                    )
                    nc.sync.dma_start(
                        bc[:, 512:1024], dd2[None, 512:1024].to_broadcast([DH, 512])
                    )
                    nc.vector.tensor_mul(
                        oT[pr][0:DH, qs], oTu[pr][0:DH, 0, qs], bc[:, 0:512]
                    )
                    o1 = nrm.tile([DH, 512], R32, name="o1", tag="o1")
                    nc.vector.tensor_mul(o1, oTu[pr][0:DH, 1, qs], bc[:, 512:1024])
                    nc.sync.dma_start(oT[pr][DH : 2 * DH, qs], o1)

                # proj(1) delayed by two q blocks (normalization drained)
                if qi == 2:
                    for tb1 in range(4, 8):
                        emit_proj_one(tb1, "sp")
            for tb1 in range(12, 16):
                emit_proj_one(tb1, "sp", use_act=True)


def build_bass():
    nc = bacc.Bacc("TRN2", target_bir_lowering=False, debug=False, num_devices=8)
    xT = nc.dram_tensor("xT", [C, T], R32, kind="ExternalInput").ap()
    wq = nc.dram_tensor("wq", [C, 2 * P], R32, kind="ExternalInput").ap()
    wk = nc.dram_tensor("wk", [C, 2 * P], R32, kind="ExternalInput").ap()
    wv = nc.dram_tensor("wv", [C, 2 * P], R32, kind="ExternalInput").ap()
    wo = nc.dram_tensor("wo", [2 * P, C], R32, kind="ExternalInput").ap()
    tri = nc.dram_tensor("tri", [P, P], F16, kind="ExternalInput").ap()
    vones = nc.dram_tensor(
        "vones", [P, NKT, HPC, 1], F16, kind="ExternalInput"
    ).ap()
    out = nc.dram_tensor("out", [T, C], F32, kind="ExternalOutput").ap()
    with tile.TileContext(nc) as tc:
        _body(tc, nc, xT, wq, wk, wv, wo, tri, vones, out)
    nc.compile()
    return nc


def _fp32r(a):
    """Round fp32 to fp32r format: 11-bit mantissa, low 12 bits zero (RTNE)."""
    u = np.ascontiguousarray(a, dtype=np.float32).view(np.uint32)
    r = (u + 0x7FF + ((u >> 12) & 1)) & np.uint32(0xFFFFF000)
    return np.ascontiguousarray(r.view(np.float32))


def make_in_maps(x, w_qkv, w_out):
    """Host-side sharding: returns the 8 per-core input dicts."""
    x = np.ascontiguousarray(np.asarray(x, dtype=np.float32))
    w_qkv = np.ascontiguousarray(np.asarray(w_qkv, dtype=np.float32))
    w_out = np.ascontiguousarray(np.asarray(w_out, dtype=np.float32))
    kk = np.arange(P)
    tri = (kk[None, :] >= kk[:, None]).astype(np.float32)  # [k, q]: q >= k
    xTb = [_fp32r(np.ascontiguousarray(x[b].T)) for b in range(B)]
    in_maps = []
    for c in range(8):
        b = c // 4
        g = c % 4
        h0 = HPC * g * DH  # 256*g
        in_maps.append(
            {
                "xT": xTb[b],
                "wq": _fp32r(w_qkv[:, h0 : h0 + 2 * P]),
                "wk": _fp32r(w_qkv[:, C + h0 : C + h0 + 2 * P]),
                "wv": _fp32r(w_qkv[:, 2 * C + h0 : 2 * C + h0 + 2 * P]),
                "wo": _fp32r(w_out[h0 : h0 + 2 * P, :]),
                "tri": np.ascontiguousarray(tri.astype(np.float16)),
                "vones": np.ones((P, NKT, HPC, 1), dtype=np.float16),
            }
        )
    return in_maps


_NC_CACHE = None
LAST_RESULTS = None  # BassKernelResults of the most recent run (for profiling)
TRACE = False


def kernel(x, w_qkv, w_out):
    global _NC_CACHE, LAST_RESULTS
    if _NC_CACHE is None:
        _NC_CACHE = build_bass()
    nc = _NC_CACHE
    in_maps = make_in_maps(x, w_qkv, w_out)
    res = bass_utils.run_bass_kernel_spmd(
        nc, in_maps, core_ids=list(range(8)), trace=TRACE
    )
    LAST_RESULTS = res
    partials = [res.results[c]["out"] for c in range(8)]
    out = np.zeros((B, T, C), dtype=np.float32)
    for c in range(8):
        out[c // 4] += partials[c]
    return out


if __name__ == "__main__":
    # smoke test with random data
    rng = np.random.default_rng(0)
    x = rng.standard_normal((B, T, C), dtype=np.float32)
    w_qkv = rng.standard_normal((C, 3 * C), dtype=np.float32) / np.sqrt(C)
    w_out = rng.standard_normal((C, C), dtype=np.float32) / np.sqrt(C)
    o = kernel(x, w_qkv, w_out)
    print(o.shape, o.dtype)


# revision 68
# speedup vs baseline: 1.0181x; 1.0181x over previous
"""Trainium2 Bass kernel for causal multi-head attention block.

Reference computation (fp32):
    qkv = x @ w_qkv;  q,k,v = split(qkv)
    attn = softmax(causal_mask(q k^T / sqrt(64)))
    out  = (attn @ v reassembled) @ w_out

Sharding over 8 NeuronCores: core c handles batch b = c//4 and heads
4*(c%4) .. 4*(c%4)+3 (4 of 16 heads).  Each core computes the rank-256
partial product of the output projection restricted to its heads'
channels; the host sums the 4 partials per batch.

Projections (x @ w_qkv, out @ w_out) run as float32r (fp32 with 11-bit
mantissa, full-rate PE mode); the attention inner loops (q k^T, P v) run
in fp16, which enables fast weight load and full-rate K=64 row-group
packing.  Softmax skips the max-subtraction (logits are O(10), fp32 exp
is safe); denominators ride along as a fused 65th lhsT column, and the
reciprocal runs on a [128,8] reshape via a DRAM round trip with a
partition-broadcast DMA.  Measured ~195-205us per core on TRN2
(scale-relative max err ~3e-4 vs the fp32 reference).
"""

import sys

for _p in ("/opt/trn_rl_repo", "/root/.axon_site/_ro/trn_rl_repo"):
    if _p not in sys.path:
        sys.path.append(_p)

import numpy as np

import concourse.bass as bass
import concourse.mybir as mybir
import concourse.tile as tile
from concourse import bacc, bass_utils

P = 128
B, T, C = 2, 2048, 1024
HPC = 4            # heads per core
DH = 64            # head dim
KT = C // P        # 8 contraction tiles over d_model
NQB = T // 512     # 4 query blocks of 512
NKT = T // P       # 16 key tiles of 128
F32 = mybir.dt.float32
R32 = mybir.dt.float32r
F16 = mybir.dt.float16
EXP = mybir.ActivationFunctionType.Exp
SCALE = 1.0 / 8.0  # 1/sqrt(DH)


def _body(tc, nc, xT, wq, wk, wv, wo, tri, vones, out):
    with tc.tile_pool(name="const", bufs=1) as cpool:
        wq_sb = cpool.tile([P, KT, 2 * P], R32, name="wq_sb")
        wk_sb = cpool.tile([P, KT, 2 * P], R32, name="wk_sb")
        wv_sb = cpool.tile([P, KT, 2 * P], R32, name="wv_sb")
        wo_sb = cpool.tile([P, 2, C], R32, name="wo_sb")
        tri_sb = cpool.tile([P, P], F16, name="tri_sb")
        # halves, interleaved with the first x tile, so the first
        # accumulation chain starts early; bulky later-phase constants go
        # through the gpsimd (SWDGE) queue so they don't delay the critical
        # path.
        wqv = wq.rearrange("(kt p) n -> p kt n", p=P)
        wkv = wk.rearrange("(kt p) n -> p kt n", p=P)
        wvv = wv.rearrange("(kt p) n -> p kt n", p=P)
        nc.sync.dma_start(wq_sb[:, 0:1], wqv[:, 0:1])
        nc.sync.dma_start(wq_sb[:, 1:4], wqv[:, 1:4])
        nc.gpsimd.dma_start(wo_sb, wo.rearrange("(g p) n -> p g n", p=P))
        nc.gpsimd.dma_start(tri_sb, tri)

        # preload the exp ACT table set during the startup DMA window
        warm = cpool.tile([1, 2], F32, name="warm")
        nc.vector.memset(warm, 0.0)
        nc.scalar.activation(warm, warm, EXP, scale=1.0)

        # persistent stores
        qT = [cpool.tile([P, T], F16, name=f"qT{pr}") for pr in range(2)]
        kT = [cpool.tile([P, T], F16, name=f"kT{pr}") for pr in range(2)]
        # v with a fused ones column: [T-part, ktile, head, 65]
        vS = cpool.tile([P, NKT, HPC, DH + 1], F16, name="vS")
        nc.gpsimd.dma_start(vS[:, :, :, DH : DH + 1], vones)
        oT = [cpool.tile([P, T], R32, name=f"oT{pr}") for pr in range(2)]
        oTu = [cpool.tile([DH + 1, 2, T], F32, name=f"oTu{pr}") for pr in range(2)]

        # ---------- phase 1: q/k/v projections ----------
        xTv = xT.rearrange("(kt p) t -> p kt t", p=P)
        with (
            tc.tile_pool(name="xt", bufs=3) as xpool,
            tc.tile_pool(name="ps1", bufs=2, space="PSUM") as ps1,
            tc.tile_pool(name="vps", bufs=2, space="PSUM") as vps,
        ):
            for tb5 in range(NQB):
                xt = xpool.tile([P, KT, 512], R32, name="xt")
                if tb5 == 0:
                    # fine-grained first chunks: the q chains start after
                    # ~0.4MB of DMA; k/v weights stream in behind
                    nc.sync.dma_start(xt[:, 0:1, :], xTv[:, 0:1, 0:512])
                    nc.sync.dma_start(xt[:, 1:4, :], xTv[:, 1:4, 0:512])
                    nc.sync.dma_start(wq_sb[:, 4:8], wqv[:, 4:8])
                    nc.sync.dma_start(xt[:, 4:8, :], xTv[:, 4:8, 0:512])
                    nc.sync.dma_start(wk_sb[:, 0:4], wkv[:, 0:4])
                    nc.sync.dma_start(wk_sb[:, 4:8], wkv[:, 4:8])
                    nc.sync.dma_start(wv_sb, wvv)
                else:
                    nc.sync.dma_start(
                        xt, xTv[:, :, tb5 * 512 : (tb5 + 1) * 512]
                    )
                qps = [ps1.tile([P, 512], F32, name=f"qp{pr}", tag=f"qp{pr}", bufs=1) for pr in range(2)]
                kps = [ps1.tile([P, 512], F32, name=f"kp{pr}", tag=f"kp{pr}", bufs=1) for pr in range(2)]
                for kt in range(KT):
                    for pr in range(2):
                        nc.tensor.matmul(
                            qps[pr],
                            wq_sb[:, kt, pr * P : (pr + 1) * P],
                            xt[:, kt, :],
                            start=(kt == 0),
                            stop=(kt == KT - 1),
                        )
                for kt in range(KT):
                    for pr in range(2):
                        nc.tensor.matmul(
                            kps[pr],
                            wk_sb[:, kt, pr * P : (pr + 1) * P],
                            xt[:, kt, :],
                            start=(kt == 0),
                            stop=(kt == KT - 1),
                        )
                for pr in range(2):
                    nc.vector.tensor_copy(qT[pr][:, tb5 * 512 : (tb5 + 1) * 512], qps[pr])
                    nc.vector.tensor_copy(kT[pr][:, tb5 * 512 : (tb5 + 1) * 512], kps[pr])
                for sub in range(4):
                    tb1 = tb5 * 4 + sub
                    vp = vps.tile([P, 2 * P], F32, name="vp")
                    for kt in range(KT):
                        nc.tensor.matmul(
                            vp,
                            xt[:, kt, sub * P : (sub + 1) * P],
                            wv_sb[:, kt, :],
                            start=(kt == 0),
                            stop=(kt == KT - 1),
                        )
                    nc.vector.tensor_copy(
                        vS[:, tb1, :, 0:DH],
                        vp.rearrange("p (h d) -> p h d", d=DH),
                    )

        # ---------- phase 2: attention ----------
        # S^T blocks: [k-tile(128) x q-block(<=512)] per head; 2 heads packed
        # on PE row groups (contraction K=64 each).  exp via ScalarE.
        # O'^T = [v | 1]-style accumulation: O rows + denominator row in the
        # same PSUM bank, relying on per-element has_written semantics
        # (single start=True matmul per bank, everything else start=False).
        with (
            tc.tile_pool(name="sps", bufs=3, space="PSUM") as sps,
            tc.tile_pool(name="ops", bufs=1, space="PSUM") as ops,
            tc.tile_pool(name="ptp", bufs=3) as ptp,
            tc.tile_pool(name="nrm", bufs=2) as nrm,
            tc.tile_pool(name="dsc", bufs=2, space="DRAM") as dsc,
            tc.tile_pool(name="osb", bufs=3) as osb,
        ):
            def emit_proj_one(tb1, tag, use_act=False):
                # output projection for one T block (both pairs' oT final).
                pp = sps.tile([P, 1024], F32, name="pp", tag=tag)
                for cb in range(2):
                    for pr in range(2):
                        nc.tensor.matmul(
                            pp[:, cb * 512 : (cb + 1) * 512],
                            oT[pr][:, tb1 * P : (tb1 + 1) * P],
                            wo_sb[:, pr, cb * 512 : (cb + 1) * 512],
                            start=(pr == 0),
                            stop=(pr == 1),
                        )
                ot = osb.tile([P, 1024], F32, name="ot")
                if use_act:
                    nc.scalar.copy(ot[:, 0:512], pp[:, 0:512])
                else:
                    nc.vector.tensor_copy(ot[:, 0:512], pp[:, 0:512])
                nc.vector.tensor_copy(ot[:, 512:1024], pp[:, 512:1024])
                nc.sync.dma_start(out[tb1 * P : (tb1 + 1) * P, :], ot)

            qb_order = [1, 0, 2, 3]
            for qi, qb in enumerate(qb_order):
                for pr in range(2):
                    op = ops.tile([P, 1024], F32, name="op", tag="op")
                    nk = 4 * qb + 4

                    def geom(j):
                        r = j - 4 * qb
                        width = 512 - r * P if r >= 0 else 512
                        col0 = r * P if r >= 0 else 0
                        return r, width, col0

                    def emit_o(j, pts):
                        _, width, col0 = geom(j)
                        pt = pts.pop(j)
                        for h in range(2):
                            nc.tensor.matmul(
                                op[0 : DH + 1, h * 512 + col0 : (h + 1) * 512],
                                vS[:, j, pr * 2 + h, :],
                                pt[:, h * 512 : h * 512 + width],
                                start=(j == 0),
                                stop=(j == nk - 1),
                                skip_group_check=True,
                            )

                    pts = {}
                    for j in range(nk):
                        r, width, col0 = geom(j)
                        qoff = qb * 512 + col0
                        sp_ = sps.tile([P, 1024], F32, name="sp_", tag="sp")
                        for h in range(2):
                            nc.tensor.matmul(
                                sp_[:, h * 512 : h * 512 + width],
                                kT[pr][h * DH : (h + 1) * DH, j * P : (j + 1) * P],
                                qT[pr][h * DH : (h + 1) * DH, qoff : qoff + width],
                                start=True,
                                stop=True,
                            )
                        pt = ptp.tile([P, 1024], F16, name="pt")
                        s3 = sp_.rearrange("p (h w) -> p h w", h=2)[:, :, 0:width]
                        p3 = pt.rearrange("p (h w) -> p h w", h=2)[:, :, 0:width]
                        nc.scalar.activation(p3, s3, EXP, scale=SCALE)
                        if r >= 0:
                            # triangular mask on the first 128 valid columns
                            for h in range(2):
                                nc.vector.tensor_mul(
                                    pt[:, h * 512 : h * 512 + P],
                                    pt[:, h * 512 : h * 512 + P],
                                    tri_sb,
                                )
                        pts[j] = pt
                        # software pipeline: O' for step j-1 after S/exp of j
                        if j > 0:
                            emit_o(j - 1, pts)
                    emit_o(nk - 1, pts)

                    # fast unnormalized evacuation: frees the op PSUM slot
                    # ~1.4us after the chain; the reciprocal/broadcast chain
                    # then runs against SBUF off the critical resource.
                    qs = slice(qb * 512, (qb + 1) * 512)
                    nc.vector.tensor_copy(
                        oTu[pr][:, :, qs],
                        op.rearrange("p (h w) -> p h w", h=2)[0 : DH + 1],
                    )
                    dd = dsc.tile([1024], F32, name="dd", tag="dd")
                    nc.sync.dma_start(
                        dd.rearrange("(h w) -> h w", h=2)[None],
                        oTu[pr][DH : DH + 1, :, qs],
                    )
                    rsh = nrm.tile([P, 8], F32, name="rsh", tag="rsh")
                    nc.sync.dma_start(rsh, dd.rearrange("(p c) -> p c", p=P))
                    rr = nrm.tile([P, 8], F32, name="rr", tag="rr")
                    nc.vector.reciprocal(rr, rsh)
                    dd2 = dsc.tile([1024], F32, name="dd2", tag="dd2")
                    nc.sync.dma_start(dd2.rearrange("(p c) -> p c", p=P), rr)
                    bc = nrm.tile([DH, 1024], F32, name="bc", tag="bc")
                    nc.sync.dma_start(
                        bc[:, 0:512], dd2[None, 0:512].to_broadcast([DH, 512])
                    )
                    nc.sync.dma_start(
                        bc[:, 512:1024], dd2[None, 512:1024].to_broadcast([DH, 512])
                    )
                    nc.vector.tensor_mul(
                        oT[pr][0:DH, qs], oTu[pr][0:DH, 0, qs], bc[:, 0:512]
                    )
                    o1 = nrm.tile([DH, 512], R32, name="o1", tag="o1")
                    nc.vector.tensor_mul(o1, oTu[pr][0:DH, 1, qs], bc[:, 512:1024])
                    nc.sync.dma_start(oT[pr][DH : 2 * DH, qs], o1)

                # projection delayed by two q blocks so its normalization
                # chain has fully drained (no PE stall on in-order issue)
                if qi > 1:
                    pq = qb_order[qi - 2]
                    for tb1 in range(pq * 4, pq * 4 + 4):
                        emit_proj_one(tb1, "sp")
            for pq in qb_order[-2:]:
                for tb1 in range(pq * 4, pq * 4 + 4):
                    emit_proj_one(tb1, "sp", use_act=True)


def build_bass():
    nc = bacc.Bacc("TRN2", target_bir_lowering=False, debug=False, num_devices=8)
    xT = nc.dram_tensor("xT", [C, T], R32, kind="ExternalInput").ap()
    wq = nc.dram_tensor("wq", [C, 2 * P], R32, kind="ExternalInput").ap()
    wk = nc.dram_tensor("wk", [C, 2 * P], R32, kind="ExternalInput").ap()
    wv = nc.dram_tensor("wv", [C, 2 * P], R32, kind="ExternalInput").ap()
    wo = nc.dram_tensor("wo", [2 * P, C], R32, kind="ExternalInput").ap()
    tri = nc.dram_tensor("tri", [P, P], F16, kind="ExternalInput").ap()
    vones = nc.dram_tensor(
        "vones", [P, NKT, HPC, 1], F16, kind="ExternalInput"
    ).ap()
    out = nc.dram_tensor("out", [T, C], F32, kind="ExternalOutput").ap()
    with tile.TileContext(nc) as tc:
        _body(tc, nc, xT, wq, wk, wv, wo, tri, vones, out)
    nc.compile()
    return nc


def _fp32r(a):
    """Round fp32 to fp32r format: 11-bit mantissa, low 12 bits zero (RTNE)."""
    u = np.ascontiguousarray(a, dtype=np.float32).view(np.uint32)
    r = (u + 0x7FF + ((u >> 12) & 1)) & np.uint32(0xFFFFF000)
    return np.ascontiguousarray(r.view(np.float32))


def make_in_maps(x, w_qkv, w_out):
    """Host-side sharding: returns the 8 per-core input dicts."""
    x = np.ascontiguousarray(np.asarray(x, dtype=np.float32))
    w_qkv = np.ascontiguousarray(np.asarray(w_qkv, dtype=np.float32))
    w_out = np.ascontiguousarray(np.asarray(w_out, dtype=np.float32))
    kk = np.arange(P)
    tri = (kk[None, :] >= kk[:, None]).astype(np.float32)  # [k, q]: q >= k
    xTb = [_fp32r(np.ascontiguousarray(x[b].T)) for b in range(B)]
    in_maps = []
    for c in range(8):
        b = c // 4
        g = c % 4
        h0 = HPC * g * DH  # 256*g
        in_maps.append(
            {
                "xT": xTb[b],
                "wq": _fp32r(w_qkv[:, h0 : h0 + 2 * P]),
                "wk": _fp32r(w_qkv[:, C + h0 : C + h0 + 2 * P]),
                "wv": _fp32r(w_qkv[:, 2 * C + h0 : 2 * C + h0 + 2 * P]),
                "wo": _fp32r(w_out[h0 : h0 + 2 * P, :]),
                "tri": np.ascontiguousarray(tri.astype(np.float16)),
                "vones": np.ones((P, NKT, HPC, 1), dtype=np.float16),
            }
        )
    return in_maps


_NC_CACHE = None
LAST_RESULTS = None  # BassKernelResults of the most recent run (for profiling)
TRACE = False


def kernel(x, w_qkv, w_out):
    global _NC_CACHE, LAST_RESULTS
    if _NC_CACHE is None:
        _NC_CACHE = build_bass()
    nc = _NC_CACHE
    in_maps = make_in_maps(x, w_qkv, w_out)
    res = bass_utils.run_bass_kernel_spmd(
        nc, in_maps, core_ids=list(range(8)), trace=TRACE
    )
    LAST_RESULTS = res
    partials = [res.results[c]["out"] for c in range(8)]
    out = np.zeros((B, T, C), dtype=np.float32)
    for c in range(8):
        out[c // 4] += partials[c]
    return out


if __name__ == "__main__":
    # smoke test with random data
    rng = np.random.default_rng(0)
    x = rng.standard_normal((B, T, C), dtype=np.float32)
    w_qkv = rng.standard_normal((C, 3 * C), dtype=np.float32) / np.sqrt(C)
    w_out = rng.standard_normal((C, C), dtype=np.float32) / np.sqrt(C)
    o = kernel(x, w_qkv, w_out)
    print(o.shape, o.dtype)


# revision 69
# speedup vs baseline: 1.0191x; 1.0010x over previous
"""Trainium2 Bass kernel for causal multi-head attention block.

Reference computation (fp32):
    qkv = x @ w_qkv;  q,k,v = split(qkv)
    attn = softmax(causal_mask(q k^T / sqrt(64)))
    out  = (attn @ v reassembled) @ w_out

Sharding over 8 NeuronCores: core c handles batch b = c//4 and heads
4*(c%4) .. 4*(c%4)+3 (4 of 16 heads).  Each core computes the rank-256
partial product of the output projection restricted to its heads'
channels; the host sums the 4 partials per batch.

Projections (x @ w_qkv, out @ w_out) run as float32r (fp32 with 11-bit
mantissa, full-rate PE mode); the attention inner loops (q k^T, P v) run
in fp16, which enables fast weight load and full-rate K=64 row-group
packing.  Softmax skips the max-subtraction (logits are O(10), fp32 exp
is safe); denominators ride along as a fused 65th lhsT column, and the
reciprocal runs on a [128,8] reshape via a DRAM round trip with a
partition-broadcast DMA.  Measured ~195-205us per core on TRN2
(scale-relative max err ~3e-4 vs the fp32 reference).
"""

import sys

for _p in ("/opt/trn_rl_repo", "/root/.axon_site/_ro/trn_rl_repo"):
    if _p not in sys.path:
        sys.path.append(_p)

import numpy as np

import concourse.bass as bass
import concourse.mybir as mybir
import concourse.tile as tile
from concourse import bacc, bass_utils

P = 128
B, T, C = 2, 2048, 1024
HPC = 4            # heads per core
DH = 64            # head dim
KT = C // P        # 8 contraction tiles over d_model
NQB = T // 512     # 4 query blocks of 512
NKT = T // P       # 16 key tiles of 128
F32 = mybir.dt.float32
R32 = mybir.dt.float32r
F16 = mybir.dt.float16
EXP = mybir.ActivationFunctionType.Exp
SCALE = 1.0 / 8.0  # 1/sqrt(DH)


def _body(tc, nc, xT, wq, wk, wv, wo, tri, vones, out):
    with tc.tile_pool(name="const", bufs=1) as cpool:
        wq_sb = cpool.tile([P, KT, 2 * P], R32, name="wq_sb")
        wk_sb = cpool.tile([P, KT, 2 * P], R32, name="wk_sb")
        wv_sb = cpool.tile([P, KT, 2 * P], R32, name="wv_sb")
        wo_sb = cpool.tile([P, 2, C], R32, name="wo_sb")
        tri_sb = cpool.tile([P, P], F16, name="tri_sb")
        # halves, interleaved with the first x tile, so the first
        # accumulation chain starts early; bulky later-phase constants go
        # through the gpsimd (SWDGE) queue so they don't delay the critical
        # path.
        wqv = wq.rearrange("(kt p) n -> p kt n", p=P)
        wkv = wk.rearrange("(kt p) n -> p kt n", p=P)
        wvv = wv.rearrange("(kt p) n -> p kt n", p=P)
        nc.sync.dma_start(wq_sb[:, 0:1], wqv[:, 0:1])
        nc.sync.dma_start(wq_sb[:, 1:4], wqv[:, 1:4])
        nc.gpsimd.dma_start(wo_sb, wo.rearrange("(g p) n -> p g n", p=P))
        nc.gpsimd.dma_start(tri_sb, tri)

        # preload the exp ACT table set during the startup DMA window
        warm = cpool.tile([1, 2], F32, name="warm")
        nc.vector.memset(warm, 0.0)
        nc.scalar.activation(warm, warm, EXP, scale=1.0)

        # persistent stores
        qT = [cpool.tile([P, T], F16, name=f"qT{pr}") for pr in range(2)]
        kT = [cpool.tile([P, T], F16, name=f"kT{pr}") for pr in range(2)]
        # v with a fused ones column: [T-part, ktile, head, 65]
        vS = cpool.tile([P, NKT, HPC, DH + 1], F16, name="vS")
        nc.gpsimd.dma_start(vS[:, :, :, DH : DH + 1], vones)
        oT = [cpool.tile([P, T], R32, name=f"oT{pr}") for pr in range(2)]
        oTu = [cpool.tile([DH + 1, 2, T], F32, name=f"oTu{pr}") for pr in range(2)]

        # ---------- phase 1: q/k/v projections ----------
        xTv = xT.rearrange("(kt p) t -> p kt t", p=P)
        with (
            tc.tile_pool(name="xt", bufs=3) as xpool,
            tc.tile_pool(name="ps1", bufs=2, space="PSUM") as ps1,
            tc.tile_pool(name="vps", bufs=2, space="PSUM") as vps,
        ):
            for tb5 in range(NQB):
                xt = xpool.tile([P, KT, 512], R32, name="xt")
                if tb5 == 0:
                    # fine-grained first chunks: the q chains start after
                    # ~0.4MB of DMA; k/v weights stream in behind
                    nc.sync.dma_start(xt[:, 0:1, :], xTv[:, 0:1, 0:512])
                    nc.sync.dma_start(xt[:, 1:4, :], xTv[:, 1:4, 0:512])
                    nc.sync.dma_start(wq_sb[:, 4:8], wqv[:, 4:8])
                    nc.sync.dma_start(xt[:, 4:8, :], xTv[:, 4:8, 0:512])
                    nc.sync.dma_start(wk_sb[:, 0:4], wkv[:, 0:4])
                    nc.sync.dma_start(wk_sb[:, 4:8], wkv[:, 4:8])
                    nc.sync.dma_start(wv_sb, wvv)
                else:
                    nc.sync.dma_start(
                        xt, xTv[:, :, tb5 * 512 : (tb5 + 1) * 512]
                    )
                qps = [ps1.tile([P, 512], F32, name=f"qp{pr}", tag=f"qp{pr}", bufs=2) for pr in range(2)]
                kps = [ps1.tile([P, 512], F32, name=f"kp{pr}", tag=f"kp{pr}", bufs=1) for pr in range(2)]
                for kt in range(KT):
                    for pr in range(2):
                        nc.tensor.matmul(
                            qps[pr],
                            wq_sb[:, kt, pr * P : (pr + 1) * P],
                            xt[:, kt, :],
                            start=(kt == 0),
                            stop=(kt == KT - 1),
                        )
                for kt in range(KT):
                    for pr in range(2):
                        nc.tensor.matmul(
                            kps[pr],
                            wk_sb[:, kt, pr * P : (pr + 1) * P],
                            xt[:, kt, :],
                            start=(kt == 0),
                            stop=(kt == KT - 1),
                        )
                for pr in range(2):
                    nc.vector.tensor_copy(qT[pr][:, tb5 * 512 : (tb5 + 1) * 512], qps[pr])
                    nc.vector.tensor_copy(kT[pr][:, tb5 * 512 : (tb5 + 1) * 512], kps[pr])
                for sub in range(4):
                    tb1 = tb5 * 4 + sub
                    vp = vps.tile([P, 2 * P], F32, name="vp")
                    for kt in range(KT):
                        nc.tensor.matmul(
                            vp,
                            xt[:, kt, sub * P : (sub + 1) * P],
                            wv_sb[:, kt, :],
                            start=(kt == 0),
                            stop=(kt == KT - 1),
                        )
                    nc.vector.tensor_copy(
                        vS[:, tb1, :, 0:DH],
                        vp.rearrange("p (h d) -> p h d", d=DH),
                    )

        # ---------- phase 2: attention ----------
        # S^T blocks: [k-tile(128) x q-block(<=512)] per head; 2 heads packed
        # on PE row groups (contraction K=64 each).  exp via ScalarE.
        # O'^T = [v | 1]-style accumulation: O rows + denominator row in the
        # same PSUM bank, relying on per-element has_written semantics
        # (single start=True matmul per bank, everything else start=False).
        with (
            tc.tile_pool(name="sps", bufs=3, space="PSUM") as sps,
            tc.tile_pool(name="ops", bufs=1, space="PSUM") as ops,
            tc.tile_pool(name="ptp", bufs=3) as ptp,
            tc.tile_pool(name="nrm", bufs=2) as nrm,
            tc.tile_pool(name="dsc", bufs=2, space="DRAM") as dsc,
            tc.tile_pool(name="osb", bufs=3) as osb,
        ):
            def emit_proj_one(tb1, tag, use_act=False):
                # output projection for one T block (both pairs' oT final).
                pp = sps.tile([P, 1024], F32, name="pp", tag=tag)
                for cb in range(2):
                    for pr in range(2):
                        nc.tensor.matmul(
                            pp[:, cb * 512 : (cb + 1) * 512],
                            oT[pr][:, tb1 * P : (tb1 + 1) * P],
                            wo_sb[:, pr, cb * 512 : (cb + 1) * 512],
                            start=(pr == 0),
                            stop=(pr == 1),
                        )
                ot = osb.tile([P, 1024], F32, name="ot")
                if use_act:
                    nc.scalar.copy(ot[:, 0:512], pp[:, 0:512])
                else:
                    nc.vector.tensor_copy(ot[:, 0:512], pp[:, 0:512])
                nc.vector.tensor_copy(ot[:, 512:1024], pp[:, 512:1024])
                nc.sync.dma_start(out[tb1 * P : (tb1 + 1) * P, :], ot)

            qb_order = [1, 0, 2, 3]
            for qi, qb in enumerate(qb_order):
                for pr in range(2):
                    op = ops.tile([P, 1024], F32, name="op", tag="op")
                    nk = 4 * qb + 4

                    def geom(j):
                        r = j - 4 * qb
                        width = 512 - r * P if r >= 0 else 512
                        col0 = r * P if r >= 0 else 0
                        return r, width, col0

                    def emit_o(j, pts):
                        _, width, col0 = geom(j)
                        pt = pts.pop(j)
                        for h in range(2):
                            nc.tensor.matmul(
                                op[0 : DH + 1, h * 512 + col0 : (h + 1) * 512],
                                vS[:, j, pr * 2 + h, :],
                                pt[:, h * 512 : h * 512 + width],
                                start=(j == 0),
                                stop=(j == nk - 1),
                                skip_group_check=True,
                            )

                    pts = {}
                    for j in range(nk):
                        r, width, col0 = geom(j)
                        qoff = qb * 512 + col0
                        sp_ = sps.tile([P, 1024], F32, name="sp_", tag="sp")
                        for h in range(2):
                            nc.tensor.matmul(
                                sp_[:, h * 512 : h * 512 + width],
                                kT[pr][h * DH : (h + 1) * DH, j * P : (j + 1) * P],
                                qT[pr][h * DH : (h + 1) * DH, qoff : qoff + width],
                                start=True,
                                stop=True,
                            )
                        pt = ptp.tile([P, 1024], F16, name="pt")
                        s3 = sp_.rearrange("p (h w) -> p h w", h=2)[:, :, 0:width]
                        p3 = pt.rearrange("p (h w) -> p h w", h=2)[:, :, 0:width]
                        nc.scalar.activation(p3, s3, EXP, scale=SCALE)
                        if r >= 0:
                            # triangular mask on the first 128 valid columns
                            for h in range(2):
                                nc.vector.tensor_mul(
                                    pt[:, h * 512 : h * 512 + P],
                                    pt[:, h * 512 : h * 512 + P],
                                    tri_sb,
                                )
                        pts[j] = pt
                        # software pipeline: O' for step j-1 after S/exp of j
                        if j > 0:
                            emit_o(j - 1, pts)
                    emit_o(nk - 1, pts)

                    # fast unnormalized evacuation: frees the op PSUM slot
                    # ~1.4us after the chain; the reciprocal/broadcast chain
                    # then runs against SBUF off the critical resource.
                    qs = slice(qb * 512, (qb + 1) * 512)
                    nc.vector.tensor_copy(
                        oTu[pr][:, :, qs],
                        op.rearrange("p (h w) -> p h w", h=2)[0 : DH + 1],
                    )
                    dd = dsc.tile([1024], F32, name="dd", tag="dd")
                    nc.sync.dma_start(
                        dd.rearrange("(h w) -> h w", h=2)[None],
                        oTu[pr][DH : DH + 1, :, qs],
                    )
                    rsh = nrm.tile([P, 8], F32, name="rsh", tag="rsh")
                    nc.sync.dma_start(rsh, dd.rearrange("(p c) -> p c", p=P))
                    rr = nrm.tile([P, 8], F32, name="rr", tag="rr")
                    nc.vector.reciprocal(rr, rsh)
                    dd2 = dsc.tile([1024], F32, name="dd2", tag="dd2")
                    nc.sync.dma_start(dd2.rearrange("(p c) -> p c", p=P), rr)
                    bc = nrm.tile([DH, 1024], F32, name="bc", tag="bc")
                    nc.sync.dma_start(
                        bc[:, 0:512], dd2[None, 0:512].to_broadcast([DH, 512])
                    )
                    nc.sync.dma_start(
                        bc[:, 512:1024], dd2[None, 512:1024].to_broadcast([DH, 512])
                    )
                    nc.vector.tensor_mul(
                        oT[pr][0:DH, qs], oTu[pr][0:DH, 0, qs], bc[:, 0:512]
                    )
                    o1 = nrm.tile([DH, 512], R32, name="o1", tag="o1")
                    nc.vector.tensor_mul(o1, oTu[pr][0:DH, 1, qs], bc[:, 512:1024])
                    nc.sync.dma_start(oT[pr][DH : 2 * DH, qs], o1)

                # projection delayed by two q blocks so its normalization
                # chain has fully drained (no PE stall on in-order issue)
                if qi > 1:
                    pq = qb_order[qi - 2]
                    for tb1 in range(pq * 4, pq * 4 + 4):
                        emit_proj_one(tb1, "sp")
            for pq in qb_order[-2:]:
                for tb1 in range(pq * 4, pq * 4 + 4):
                    emit_proj_one(tb1, "sp", use_act=True)


def build_bass():
    nc = bacc.Bacc("TRN2", target_bir_lowering=False, debug=False, num_devices=8)
    xT = nc.dram_tensor("xT", [C, T], R32, kind="ExternalInput").ap()
    wq = nc.dram_tensor("wq", [C, 2 * P], R32, kind="ExternalInput").ap()
    wk = nc.dram_tensor("wk", [C, 2 * P], R32, kind="ExternalInput").ap()
    wv = nc.dram_tensor("wv", [C, 2 * P], R32, kind="ExternalInput").ap()
    wo = nc.dram_tensor("wo", [2 * P, C], R32, kind="ExternalInput").ap()
    tri = nc.dram_tensor("tri", [P, P], F16, kind="ExternalInput").ap()
    vones = nc.dram_tensor(
        "vones", [P, NKT, HPC, 1], F16, kind="ExternalInput"
    ).ap()
    out = nc.dram_tensor("out", [T, C], F32, kind="ExternalOutput").ap()
    with tile.TileContext(nc) as tc:
        _body(tc, nc, xT, wq, wk, wv, wo, tri, vones, out)
    nc.compile()
    return nc


def _fp32r(a):
    """Round fp32 to fp32r format: 11-bit mantissa, low 12 bits zero (RTNE)."""
    u = np.ascontiguousarray(a, dtype=np.float32).view(np.uint32)
    r = (u + 0x7FF + ((u >> 12) & 1)) & np.uint32(0xFFFFF000)
    return np.ascontiguousarray(r.view(np.float32))


def make_in_maps(x, w_qkv, w_out):
    """Host-side sharding: returns the 8 per-core input dicts."""
    x = np.ascontiguousarray(np.asarray(x, dtype=np.float32))
    w_qkv = np.ascontiguousarray(np.asarray(w_qkv, dtype=np.float32))
    w_out = np.ascontiguousarray(np.asarray(w_out, dtype=np.float32))
    kk = np.arange(P)
    tri = (kk[None, :] >= kk[:, None]).astype(np.float32)  # [k, q]: q >= k
    xTb = [_fp32r(np.ascontiguousarray(x[b].T)) for b in range(B)]
    in_maps = []
    for c in range(8):
        b = c // 4
        g = c % 4
        h0 = HPC * g * DH  # 256*g
        in_maps.append(
            {
                "xT": xTb[b],
                "wq": _fp32r(w_qkv[:, h0 : h0 + 2 * P]),
                "wk": _fp32r(w_qkv[:, C + h0 : C + h0 + 2 * P]),
                "wv": _fp32r(w_qkv[:, 2 * C + h0 : 2 * C + h0 + 2 * P]),
                "wo": _fp32r(w_out[h0 : h0 + 2 * P, :]),
                "tri": np.ascontiguousarray(tri.astype(np.float16)),
                "vones": np.ones((P, NKT, HPC, 1), dtype=np.float16),
            }
        )
    return in_maps


_NC_CACHE = None
LAST_RESULTS = None  # BassKernelResults of the most recent run (for profiling)
TRACE = False


def kernel(x, w_qkv, w_out):
    global _NC_CACHE, LAST_RESULTS
    if _NC_CACHE is None:
        _NC_CACHE = build_bass()
    nc = _NC_CACHE
    in_maps = make_in_maps(x, w_qkv, w_out)
    res = bass_utils.run_bass_kernel_spmd(
        nc, in_maps, core_ids=list(range(8)), trace=TRACE
    )
    LAST_RESULTS = res
    partials = [res.results[c]["out"] for c in range(8)]
    out = np.zeros((B, T, C), dtype=np.float32)
    for c in range(8):
        out[c // 4] += partials[c]
    return out


if __name__ == "__main__":
    # smoke test with random data
    rng = np.random.default_rng(0)
    x = rng.standard_normal((B, T, C), dtype=np.float32)
    w_qkv = rng.standard_normal((C, 3 * C), dtype=np.float32) / np.sqrt(C)
    w_out = rng.standard_normal((C, C), dtype=np.float32) / np.sqrt(C)
    o = kernel(x, w_qkv, w_out)
    print(o.shape, o.dtype)
